# revision 19
# baseline (speedup 1.0000x reference)
"""Distributed Trainium2 Bass kernel for AdaptivePseudoLabelRefinement.

Sharding: feature-dim (D=131072) sharded 8 ways for the bank-distance phase
(each core reads a 128MB slice of the 1GiB bank exactly once); batch-dim
sharded for the softmax/label phase (core c handles batch b=c).  Two tiny
AllReduces ([8,2048] partial scores, [8,16] partial d_aug^2) glue the phases.
"""

import os
import sys

import numpy as np

for _p in ("/opt/trn_rl_repo",):
    if _p not in sys.path:
        sys.path.insert(0, _p)

import concourse.bass as bass
import concourse.mybir as mybir
import concourse.tile as tile
from concourse import bacc
from concourse.bass import ds
from concourse.bass_utils import run_bass_kernel_spmd
from concourse.masks import make_identity

NCORES = 8
BANK = 2048
D = 512 * 16 * 16            # 131072
DS = D // NCORES             # 16384 per-core feature shard
KC = DS // 128               # 128 contraction chunks per core
B, A, C, H, W = 8, 16, 19, 128, 128
PX = H * W                   # 16384
GBLK = 16                    # pixel-block group size in phase 4 (128 blocks total)
NPASS = (PX // 128) // GBLK  # 8 passes
BIG = 1.0e30

F32 = mybir.dt.float32
F32R = mybir.dt.float32r
I32 = mybir.dt.int32
U32 = mybir.dt.uint32

USE_F32R = os.environ.get("KNN_F32R", "1") == "1"


MMDT = F32R if USE_F32R else F32


def _r(ap):
    return ap


def build(k: int):
    assert 1 <= k <= 8
    nc = bacc.Bacc(None, target_bir_lowering=False)

    q = nc.declare_dram_parameter("q", [BANK, DS], F32, isOutput=False)
    tpre = nc.declare_dram_parameter("tpre", [128, KC * B], F32, isOutput=False)
    tnorm = nc.declare_dram_parameter("tnorm", [B, 1], F32, isOutput=False)
    augp = nc.declare_dram_parameter("augp", [128, KC, B * A], F32, isOutput=False)
    sel = nc.declare_dram_parameter("sel", [B, 1], F32, isOutput=False)
    onesin = nc.declare_dram_parameter("onesin", [128, 8], F32, isOutput=False)
    cic = nc.declare_dram_parameter("cic", [128, GBLK * C], F32, isOutput=False)
    c8 = nc.declare_dram_parameter("c8", [B, B * A + 1 + A], I32, isOutput=False)
    lg = nc.declare_dram_parameter("lg", [128, 128, A, C], F32, isOutput=False)
    pl = nc.declare_dram_parameter("pl", [128, 128, C], F32, isOutput=False)
    osoft = nc.declare_dram_parameter("osoft", [128, 128, C], F32, isOutput=True)
    oref = nc.declare_dram_parameter("oref", [128, 128], F32, isOutput=True)
    DBG = os.environ.get("KNN_DBG", "0") == "1"
    if DBG:
        odbg_eq = nc.declare_dram_parameter("odbg_eq", [128, 128], I32, isOutput=True)
        odbg_cand = nc.declare_dram_parameter("odbg_cand", [128, 128], I32, isOutput=True)
        odbg_mx = nc.declare_dram_parameter("odbg_mx", [128, 128], F32, isOutput=True)

    rg = [list(range(NCORES))]

    with tile.TileContext(nc) as tc:
        with (
            tc.tile_pool(name="const", bufs=1) as cpool,
            tc.tile_pool(name="qp", bufs=2) as qpool,
            tc.tile_pool(name="sqp", bufs=2) as sqpool,
            tc.tile_pool(name="small", bufs=1) as spool,
            tc.tile_pool(name="dram", bufs=1, space="DRAM") as dpool,
        ):
            # ---------- persistent small tiles ----------
            tpre_sb = cpool.tile([128, KC * B], MMDT)
            nc.sync.dma_start(tpre_sb[:], tpre[:].bitcast(MMDT))
            ones8 = cpool.tile([128, 8], MMDT)
            nc.sync.dma_start(ones8[:], onesin[:].bitcast(MMDT))
            ident = cpool.tile([128, 128], F32)
            make_identity(nc, ident[:])

            # ---------- phase 1: scores[b, n] = ||q_n||^2 - 2 t_b . q_n (partial over D shard)
            q_r = q[:].bitcast(MMDT).rearrange("n (kc p) -> kc p n", p=128)  # [KC, 128, 2048]
            with tc.tile_pool(name="ps1", bufs=1, space="PSUM") as ps1:
                ps_score = ps1.tile([8, BANK], F32)
                ps_norm = ps1.tile([1, BANK], F32)
                for kc in range(KC):
                    qt = qpool.tile([128, BANK], MMDT, tag="qt")
                    nc.sync.dma_start(qt[:], q_r[kc])
                    sq = sqpool.tile([128, BANK], MMDT, tag="sq")
                    nc.scalar.activation(
                        sq[:], qt[:], mybir.ActivationFunctionType.Square
                    )
                    lhs_t = tpre_sb[:, kc * 8:(kc + 1) * 8]
                    st = kc == 0
                    sp = kc == KC - 1
                    for j in range(4):
                        nsl = ds(j * 512, 512)
                        nc.tensor.matmul(
                            ps_score[:, nsl], lhsT=_r(lhs_t), rhs=_r(qt[:, nsl]),
                            start=st, stop=sp,
                        )
                        nc.tensor.matmul(
                            ps_norm[:, nsl], lhsT=_r(ones8[:, 0:1]), rhs=_r(sq[:, nsl]),
                            start=st, stop=sp,
                        )

                # combine: score_sb = ps_score + bcast(ps_norm)
                normrow = spool.tile([1, BANK], F32)
                nc.scalar.activation(
                    normrow[:], ps_norm[:], mybir.ActivationFunctionType.Copy
                )
                norm_bc = spool.tile([8, BANK], F32)
                nc.gpsimd.partition_broadcast(norm_bc[:], normrow[:])
                score_sb = spool.tile([8, BANK], F32, tag="sc8")
                nc.vector.tensor_add(score_sb[:], ps_score[:], norm_bc[:])

            # AllReduce #1: full scores
            arin1 = dpool.tile([8, BANK], F32)
            arout1 = dpool.tile([8, BANK], F32)
            nc.sync.dma_start(arin1[:], score_sb[:])
            nc.gpsimd.collective_compute(
                "AllReduce", mybir.AluOpType.add,
                ins=[arin1.opt()], outs=[arout1.opt()], replica_groups=rg,
            )
            sfull = spool.tile([8, BANK], F32, tag="sc8")
            nc.sync.dma_start(sfull[:], arout1[:])

            # argmin over bank (negate in place)
            nc.vector.tensor_scalar_mul(sfull[:], sfull[:], -1.0)
            mx8 = spool.tile([8, 8], F32)
            mi8 = spool.tile([8, 8], U32)
            nc.vector.max_with_indices(mx8[:], mi8[:], sfull[:])

            # closest_dist^2 = min_score + ||t_b||^2  = tnorm - max(neg)
            tn = spool.tile([8, 1], F32)
            nc.sync.dma_start(tn[:], tnorm[:])
            cd2 = spool.tile([8, 1], F32)
            nc.vector.tensor_sub(cd2[:], tn[:], mx8[:, 0:1])

            # ---------- phase 2: gather closest shard, d_aug^2 partials ----------
            closest = cpool.tile([8, DS], F32)
            nc.gpsimd.indirect_dma_start(
                out=closest[:],
                out_offset=None,
                in_=q[:],
                in_offset=bass.IndirectOffsetOnAxis(ap=mi8[:, 0:1], axis=0),
            )
            # transpose closest into ct [128, KC*8], scaled by -2
            ct = cpool.tile([128, KC * 8], MMDT)
            with tc.tile_pool(name="ps2", bufs=2, space="PSUM") as ps2:
                for g in range(KC // 8):
                    pst = ps2.tile([128, 64], F32, tag="pst")
                    for j in range(8):
                        kc = g * 8 + j
                        nc.tensor.transpose(
                            pst[:, j * 8:(j + 1) * 8],
                            closest[:, kc * 128:(kc + 1) * 128],
                            ident[0:8, 0:8],
                        )
                    nc.scalar.activation(
                        ct[:, g * 64:(g + 1) * 64], pst[:],
                        mybir.ActivationFunctionType.Copy, scale=-2.0,
                    )

            # ||closest||^2 partial (in-place square after transposes consumed it)
            csq = spool.tile([8, 1], F32)
            nc.scalar.activation(
                closest[:], closest[:], mybir.ActivationFunctionType.Square,
                accum_out=csq[:],
            )

            # aug dots + aug norms via PE
            with tc.tile_pool(name="ps3", bufs=1, space="PSUM") as ps3:
                ps_dot = ps3.tile([8, B * A], F32)
                ps_an = ps3.tile([8, B * A], F32)
                for kc in range(KC):
                    at = qpool.tile([128, B * A], MMDT, tag="at")
                    nc.sync.dma_start(at[:], augp[:].bitcast(MMDT)[:, kc, :])
                    asq = sqpool.tile([128, B * A], MMDT, tag="asq")
                    nc.scalar.activation(
                        asq[:], at[:], mybir.ActivationFunctionType.Square
                    )
                    st = kc == 0
                    sp = kc == KC - 1
                    nc.tensor.matmul(
                        ps_dot[:], lhsT=_r(ct[:, kc * 8:(kc + 1) * 8]), rhs=_r(at[:]),
                        start=st, stop=sp,
                    )
                    nc.tensor.matmul(
                        ps_an[:], lhsT=_r(ones8[:]), rhs=_r(asq[:]),
                        start=st, stop=sp,
                    )
                # daug128[b, x] = dot + aug_norm + csq ; only x in [16b,16b+16) is real
                an_sb = spool.tile([8, B * A], F32)
                nc.scalar.activation(
                    an_sb[:], ps_an[:], mybir.ActivationFunctionType.Copy
                )
                daug = spool.tile([8, B * A], F32)
                nc.vector.tensor_add(daug[:], ps_dot[:], an_sb[:])
            nc.vector.tensor_scalar_add(daug[:], daug[:], csq[:])

            # AllReduce #2: full d_aug^2
            arin2 = dpool.tile([8, B * A], F32)
            arout2 = dpool.tile([8, B * A], F32)
            nc.sync.dma_start(arin2[:], daug[:])
            nc.gpsimd.collective_compute(
                "AllReduce", mybir.AluOpType.add,
                ins=[arin2.opt()], outs=[arout2.opt()], replica_groups=rg,
            )
            dfull = spool.tile([8, B * A], F32)
            nc.sync.dma_start(dfull[:], arout2[:])

            # ---------- phase 3: top-k selection weights ----------
            # off-diagonal-block mask: val = x - 16*b; real iff 0 <= val < 16
            c8_sb = spool.tile([8, B * A + 1 + A], I32)
            nc.sync.dma_start(c8_sb[:], c8[:])
            valf = spool.tile([8, B * A], F32)
            nc.vector.tensor_copy(valf[:], c8_sb[:, 0:B * A])
            offm = spool.tile([8, B * A], F32)
            nc.vector.tensor_scalar(
                offm[:], valf[:], -0.5, None, op0=mybir.AluOpType.is_lt
            )
            offm2 = spool.tile([8, B * A], F32)
            nc.vector.tensor_scalar(
                offm2[:], valf[:], A - 0.5, None, op0=mybir.AluOpType.is_gt
            )
            nc.vector.tensor_add(offm[:], offm[:], offm2[:])
            nc.vector.tensor_scalar_mul(offm[:], offm[:], BIG)
            dm = spool.tile([8, B * A], F32)
            nc.vector.tensor_add(dm[:], dfull[:], offm[:])
            # mask: d_aug^2 <= closest_dist^2  (garbage cols fail via +BIG)
            maskv = spool.tile([8, B * A], F32)
            nc.vector.tensor_scalar(
                maskv[:], dm[:], cd2[:], None, op0=mybir.AluOpType.is_le
            )
            # masked = maskv*dm + (1-maskv)*BIG, exactly (no cancellation)
            inv = spool.tile([8, B * A], F32)
            nc.vector.tensor_scalar(
                inv[:], maskv[:], -BIG, BIG, op0=mybir.AluOpType.mult,
                op1=mybir.AluOpType.add,
            )
            nmd = spool.tile([8, B * A], F32)
            nc.vector.tensor_mul(nmd[:], maskv[:], dm[:])
            nc.vector.tensor_add(nmd[:], nmd[:], inv[:])
            nc.vector.tensor_scalar_mul(nmd[:], nmd[:], -1.0)
            mv8 = spool.tile([8, 8], F32)
            mi8b = spool.tile([8, 8], U32)
            nc.vector.max_with_indices(mv8[:], mi8b[:], nmd[:])
            # sel_d2 = -mv8[:, :k]; valid = sel_d2 < BIG/2  <=>  mv8 > -BIG/2
            valid = spool.tile([8, 8], F32)
            nc.vector.tensor_scalar(
                valid[:], mv8[:], -BIG / 2, None, op0=mybir.AluOpType.is_gt
            )
            count = spool.tile([8, 1], F32)
            nc.vector.tensor_reduce(
                count[:], valid[:, 0:k], axis=mybir.AxisListType.X,
                op=mybir.AluOpType.add,
            )
            cnt1 = spool.tile([8, 1], F32)
            nc.vector.tensor_scalar_max(cnt1[:], count[:], 1.0)
            rec = spool.tile([8, 1], F32)
            nc.vector.reciprocal(rec[:], cnt1[:])
            wsel = spool.tile([8, 8], F32)
            nc.vector.tensor_scalar_mul(wsel[:], valid[:], rec[:])
            g0 = spool.tile([8, 1], F32)
            nc.vector.tensor_scalar(
                g0[:], count[:], 0.5, None, op0=mybir.AluOpType.is_lt
            )
            # scatter slot weights to per-augmentation weights w_a[8, A]
            iota16f = spool.tile([8, A], F32)
            nc.vector.tensor_copy(iota16f[:], c8_sb[:, B * A + 1:])
            pscale = spool.tile([8, 1], F32)
            nc.vector.tensor_copy(pscale[:], c8_sb[:, B * A:B * A + 1])
            idxf = spool.tile([8, 8], F32)
            nc.vector.tensor_copy(idxf[:], mi8b[:])
            nc.vector.tensor_scalar_sub(idxf[:], idxf[:], pscale[:])
            w_a = spool.tile([8, A], F32)
            nc.vector.memset(w_a[:], 0.0)
            for s in range(k):
                cmp = spool.tile([8, A], F32, tag="cmp")
                nc.vector.tensor_scalar(
                    cmp[:], iota16f[:], idxf[:, s:s + 1], None,
                    op0=mybir.AluOpType.is_equal,
                )
                nc.vector.tensor_scalar_mul(cmp[:], cmp[:], wsel[:, s:s + 1])
                nc.vector.tensor_add(w_a[:], w_a[:], cmp[:])

            # pick row b=core_id via one-hot sel matmul; combo = [w_a | g0]
            combo = spool.tile([8, A + 1], F32)
            nc.vector.tensor_copy(combo[:, 0:A], w_a[:])
            nc.vector.tensor_copy(combo[:, A:A + 1], g0[:])
            sel_sb = spool.tile([8, 1], F32)
            nc.sync.dma_start(sel_sb[:], sel[:])
            with tc.tile_pool(name="ps4", bufs=1, space="PSUM") as ps4:
                ps_row = ps4.tile([1, A + 1], F32)
                nc.tensor.matmul(
                    ps_row[:], lhsT=sel_sb[:], rhs=combo[:], start=True, stop=True
                )
                row_sb = spool.tile([1, A + 1], F32)
                nc.scalar.activation(
                    row_sb[:], ps_row[:], mybir.ActivationFunctionType.Copy
                )
            wbc = cpool.tile([128, A + 1], F32)
            nc.gpsimd.partition_broadcast(wbc[:], row_sb[:])

            # ---------- phase 4: softmax + weighted sum + blend + argmax ----------
            lg_r = lg[:]        # [128 p, 128 g, A, C] host-pretransposed
            pl_r = pl[:]        # [128 p, 128 g, C]
            os_r = osoft[:]
            or_r = oref[:]      # [128 p, 128 g]

            iotac = cpool.tile([128, GBLK, C], F32)
            nc.sync.dma_start(iotac[:], cic[:].rearrange("p (g c) -> p g c", c=C))

            with (
                tc.tile_pool(name="lgp", bufs=2) as lgpool,
                tc.tile_pool(name="plp", bufs=2) as plpool,
                tc.tile_pool(name="accp", bufs=2) as accpool,
            ):
                for gp in range(NPASS):
                    gsl = ds(gp * GBLK, GBLK)
                    lt = lgpool.tile([128, GBLK, A, C], F32, tag="lt")
                    nc.sync.dma_start(lt[:], lg_r[:, gsl, :, :])
                    # exp (no max-subtraction needed: |logits| small)
                    nc.scalar.activation(
                        lt[:], lt[:], mybir.ActivationFunctionType.Exp
                    )
                    s_all = accpool.tile([128, GBLK, A], F32, tag="s_all")
                    nc.vector.tensor_reduce(
                        s_all[:], lt[:], axis=mybir.AxisListType.X,
                        op=mybir.AluOpType.add,
                    )
                    coef = accpool.tile([128, GBLK, A], F32, tag="coef")
                    nc.vector.reciprocal(coef[:], s_all[:])
                    nc.vector.tensor_mul(
                        coef[:], coef[:],
                        wbc[:, 0:A].unsqueeze(1).broadcast_to([128, GBLK, A]),
                    )
                    # weight each softmax: lt *= coef  (broadcast over C)
                    nc.vector.tensor_mul(
                        lt[:], lt[:],
                        coef[:].unsqueeze(3).broadcast_to([128, GBLK, A, C]),
                    )
                    # sum over augmentations -> soft labels
                    acc = accpool.tile([128, GBLK, C], F32, tag="acc")
                    nc.vector.tensor_reduce(
                        acc[:], lt[:].rearrange("p g a c -> p g c a"),
                        axis=mybir.AxisListType.X, op=mybir.AluOpType.add,
                    )
                    # fallback blend: acc += g0 * pseudo_label
                    plt = plpool.tile([128, GBLK, C], F32, tag="plt")
                    nc.sync.dma_start(plt[:], pl_r[:, gsl, :])
                    nc.vector.tensor_scalar_mul(plt[:], plt[:], wbc[:, A:A + 1])
                    nc.vector.tensor_add(acc[:], acc[:], plt[:])
                    nc.sync.dma_start(os_r[:, gsl, :], acc[:])
                    # argmax over C (first max index)
                    mx = accpool.tile([128, GBLK], F32, tag="mx")
                    nc.vector.tensor_reduce(
                        mx[:], acc[:], axis=mybir.AxisListType.X,
                        op=mybir.AluOpType.max,
                    )
                    eq = accpool.tile([128, GBLK, C], F32, tag="eq")
                    nc.vector.tensor_tensor(
                        eq[:], acc[:],
                        mx[:].unsqueeze(2).broadcast_to([128, GBLK, C]),
                        op=mybir.AluOpType.is_ge,
                    )
                    # cand = iota + (1-eq)*BIGF, all f32 (exact for eq in {0,1})
                    nc.vector.tensor_scalar(
                        eq[:], eq[:], -1000.0, 1000.0, op0=mybir.AluOpType.mult,
                        op1=mybir.AluOpType.add,
                    )
                    nc.vector.tensor_add(eq[:], eq[:], iotac[:])
                    ridxf = accpool.tile([128, GBLK], F32, tag="ridxf")
                    nc.vector.tensor_reduce(
                        ridxf[:], eq[:], axis=mybir.AxisListType.X,
                        op=mybir.AluOpType.min,
                    )
                    nc.sync.dma_start(or_r[:, gsl], ridxf[:])
                    if DBG:
                        nc.sync.dma_start(odbg_mx[:][:, gsl], mx[:])

    nc.compile()
    return nc


_CACHE = {}


def prepare(source_queue, tgt_feat, auged_feat, auged_logits, pseudo_label, k):
    source_queue = np.asarray(source_queue, dtype=np.float32)
    tgt_feat = np.asarray(tgt_feat, dtype=np.float32)
    auged_feat = np.asarray(auged_feat, dtype=np.float32)
    auged_logits = np.asarray(auged_logits, dtype=np.float32)
    pseudo_label = np.asarray(pseudo_label, dtype=np.float32)
    k = int(np.asarray(k))

    Q2 = source_queue.reshape(BANK, D)
    T2 = tgt_feat.reshape(B, D)
    AG = auged_feat.reshape(B, A, D)
    # [b, a, c, g, p] -> per-core [p, g, a, c] / [p, g, c] layouts for 3-dim DMA APs
    LG = auged_logits.reshape(B, A, C, 128, 128)
    PL = pseudo_label.reshape(B, C, 128, 128)
    tnorm = (T2.astype(np.float64) ** 2).sum(1).astype(np.float32).reshape(B, 1)

    in_maps = []
    for c in range(NCORES):
        dsl = slice(c * DS, (c + 1) * DS)
        tpre = np.ascontiguousarray(
            (-2.0 * T2[:, dsl]).reshape(B, KC, 128).transpose(2, 1, 0)
        ).reshape(128, KC * B)
        augp = np.ascontiguousarray(
            AG[:, :, dsl].reshape(B, A, KC, 128).transpose(3, 2, 0, 1)
        ).reshape(128, KC, B * A)
        selv = np.zeros((B, 1), np.float32)
        selv[c, 0] = 1.0
        in_maps.append({
            "q": np.ascontiguousarray(Q2[:, dsl]),
            "tpre": np.ascontiguousarray(tpre),
            "tnorm": tnorm,
            "augp": np.ascontiguousarray(augp),
            "sel": selv,
            "onesin": np.ones((128, 8), np.float32),
            "cic": np.tile(np.arange(C, dtype=np.float32), (128, GBLK)).reshape(
                128, GBLK * C),
            "c8": np.concatenate([
                np.arange(B * A, dtype=np.int32)[None, :]
                - A * np.arange(B, dtype=np.int32)[:, None],
                (A * np.arange(B, dtype=np.int32))[:, None],
                np.tile(np.arange(A, dtype=np.int32), (B, 1)),
            ], axis=1),
            "lg": np.ascontiguousarray(LG[c].transpose(3, 2, 0, 1)),
            "pl": np.ascontiguousarray(PL[c].transpose(2, 1, 0)),
        })

    return in_maps, k


def kernel(**inputs):
    in_maps, k = prepare(**inputs)
    if k not in _CACHE:
        _CACHE[k] = build(k)
    nc = _CACHE[k]

    trace = os.environ.get("KNN_TRACE", "0") == "1"
    res = run_bass_kernel_spmd(
        nc, in_maps, core_ids=list(range(NCORES)), trace=trace,
        trace_cores=list(range(NCORES)) if trace else None,
    )
    if res.exec_time_ns is not None:
        print(f"HW exec time: {res.exec_time_ns} ns")
        if res.instructions_and_trace is not None:
            print(f"trace: {res.instructions_and_trace[1]}")
    # osoft [p, g, c] -> [c, g, p] -> [C, H, W];  oref [p, g] -> [g, p] -> [H, W]
    soft = np.stack([
        np.ascontiguousarray(res.results[c]["osoft"].transpose(2, 1, 0)).reshape(C, H, W)
        for c in range(NCORES)
    ])
    refined = np.stack([
        np.ascontiguousarray(res.results[c]["oref"].T).reshape(H, W).astype(np.int32)
        for c in range(NCORES)
    ])
    return refined, soft
